# revision 4
# baseline (speedup 1.0000x reference)
"""Single-head causal attention (B=8, T=2048, C=1024, H=128) on 8 trn2 cores.

Data-parallel over batch: core b computes attention for batch element b.

Per-core device algorithm (all matmuls in float32r, 1 cycle/row at N>=512):
  inputs (host-prepped): xT = x[b].T [C,T], Wq/Wk/Wv [C,H], masks, identity, ones
  1. qT = Wq.T @ xT, kT = Wk.T @ xT, vT = Wv.T @ xT       [H, T] each
  2. v = vT.T via PE transpose                            [T, H]
  3. per 512-wide q-range r, per 128-wide k-strip kt<=4r+3:
       ST[k,q] = kT[:,kt].T @ qT[:,r]    (scores, transposed)   PSUM [128,512]
       E = exp(ST/sqrt(C))  on ScalarE (scale folded into activation)
       causal mask on diagonal strips: E *= mask01 (VectorE)
     outT[r] = sum_kt v[kt].T @ E[kt]                     PSUM [H,512]
     l[r]    = sum_kt ones.T @ E[kt]   (softmax denominators, [1,512])
  4. lT via tiny matmul against ident[0:1,0:1]; recip on VectorE
  5. out[qt] = (outT.T per 128-tile via PE transpose) * recip_l   -> DRAM

No max-subtraction in softmax: |S/sqrt(C)| <= ~8 for this problem's
distribution (x,W ~ N(0,1)/N(0,1/C)), well within fp32 exp range.
"""

import numpy as np

import concourse.bacc as bacc
import concourse.mybir as mybir
import concourse.tile as tile
from concourse.bass_utils import run_bass_kernel_spmd

B, T, C, H = 8, 2048, 1024, 128
NCORES = 8
QR = 512          # q-range width (one PSUM bank)
NQR = T // QR     # 4 q-ranges
NKT = T // 128    # 16 k-strips
NCC = C // 128    # 8 contraction chunks
SCALE = 1.0 / np.sqrt(C)

F32 = mybir.dt.float32
F32R = mybir.dt.float32r


def _build_program():
    nc = bacc.Bacc("TRN2", target_bir_lowering=False, debug=False,
                   num_devices=NCORES)

    xT_d = nc.dram_tensor("xT", [C, T], F32R, kind="ExternalInput")
    Wq_d = nc.dram_tensor("Wq", [C, H], F32R, kind="ExternalInput")
    Wk_d = nc.dram_tensor("Wk", [C, H], F32R, kind="ExternalInput")
    Wv_d = nc.dram_tensor("Wv", [C, H], F32R, kind="ExternalInput")
    masks_d = nc.dram_tensor("masks", [4, 128, QR], F32, kind="ExternalInput")
    ident_d = nc.dram_tensor("ident", [128, 128], F32, kind="ExternalInput")
    ones_d = nc.dram_tensor("ones", [128, 1], F32R, kind="ExternalInput")
    out_d = nc.dram_tensor("out", [T, H], F32, kind="ExternalOutput")

    with tile.TileContext(nc) as tc:
        with (
            tc.tile_pool(name="consts", bufs=1) as consts,
            tc.tile_pool(name="xt", bufs=NCC * NQR) as xt_pool,
            tc.tile_pool(name="qkvT", bufs=1) as qkvT_pool,
            tc.tile_pool(name="vnat", bufs=NKT) as vnat_pool,
            tc.tile_pool(name="e", bufs=20) as e_pool,
            tc.tile_pool(name="osmall", bufs=1) as osmall_pool,
            tc.tile_pool(name="ofin", bufs=4) as ofin_pool,
            tc.tile_pool(name="mm512", bufs=3, space="PSUM") as mm512_pool,
            tc.tile_pool(name="acc", bufs=1, space="PSUM") as acc_pool,
            tc.tile_pool(name="trps", bufs=2, space="PSUM") as trps_pool,
            tc.tile_pool(name="ltps", bufs=1, space="PSUM") as ltps_pool,
        ):
            # ---- constants -------------------------------------------------
            wq_sb = consts.tile([128, NCC, H], F32R, tag="wq")
            wk_sb = consts.tile([128, NCC, H], F32R, tag="wk")
            wv_sb = consts.tile([128, NCC, H], F32R, tag="wv")
            for w_sb, w_d in ((wq_sb, Wq_d), (wk_sb, Wk_d), (wv_sb, Wv_d)):
                nc.sync.dma_start(
                    w_sb[:], w_d.ap().rearrange("(cc p) h -> p cc h", p=128))
            mask_sb = consts.tile([128, 4, QR], F32, tag="mask")
            nc.sync.dma_start(
                mask_sb[:], masks_d.ap().rearrange("j p f -> p j f"))
            ident_sb = consts.tile([128, 128], F32, tag="ident")
            nc.sync.dma_start(ident_sb[:], ident_d.ap())
            ones_sb = consts.tile([128, 1], F32R, tag="ones")
            nc.sync.dma_start(ones_sb[:], ones_d.ap())

            # ---- load xT as [cc][s] tiles of [128, 512] --------------------
            xt = [[None] * NQR for _ in range(NCC)]
            for s in range(NQR):
                for cc in range(NCC):
                    t_ = xt_pool.tile([128, QR], F32R, tag="xt")
                    nc.sync.dma_start(
                        t_[:],
                        xT_d.ap()[128 * cc:128 * (cc + 1), QR * s:QR * (s + 1)])
                    xt[cc][s] = t_

            # ---- stage 1: qT/kT/vT = W.T @ xT ------------------------------
            qT = qkvT_pool.tile([128, T], F32R, tag="qT")
            kT = qkvT_pool.tile([128, T], F32R, tag="kT")
            vT = qkvT_pool.tile([128, T], F32, tag="vT")
            for s in range(NQR):
                for w_sb, dst in ((wq_sb, qT), (wk_sb, kT), (wv_sb, vT)):
                    ps = mm512_pool.tile([128, QR], F32, tag="mm512")
                    for cc in range(NCC):
                        nc.tensor.matmul(
                            ps[:],
                            w_sb[:, cc, :],
                            xt[cc][s][:],
                            start=(cc == 0), stop=(cc == NCC - 1))
                    nc.scalar.copy(dst[:, QR * s:QR * (s + 1)], ps[:])

            # ---- stage 1b: v natural [T, H] via PE transpose ---------------
            v_nat = []
            for kt in range(NKT):
                ps = trps_pool.tile([128, 128], F32, tag="trps")
                nc.tensor.transpose(
                    ps[:], vT[:, 128 * kt:128 * (kt + 1)], ident_sb[:])
                vt_sb = vnat_pool.tile([128, 128], F32R, tag="vnat")
                nc.vector.tensor_copy(vt_sb[:], ps[:])
                v_nat.append(vt_sb)

            # ---- stage 2: attention per q-range ----------------------------
            outT_sb = []   # [H, 512] per r, unnormalized out, transposed
            l_sb = []      # [1, 512] per r, softmax denominators
            for r in range(NQR):
                nkt = 4 * r + 4
                e_tiles = []
                for kt in range(nkt):
                    st = mm512_pool.tile([128, QR], F32, tag="mm512")
                    nc.tensor.matmul(
                        st[:],
                        kT[:, 128 * kt:128 * (kt + 1)],
                        qT[:, QR * r:QR * (r + 1)],
                        start=True, stop=True)
                    e = e_pool.tile([128, QR], F32R, tag="e")
                    nc.scalar.activation(
                        e[:], st[:], mybir.ActivationFunctionType.Exp,
                        scale=float(SCALE))
                    j = kt - 4 * r
                    if j >= 0:
                        w = 128 * (j + 1)
                        nc.vector.tensor_mul(
                            e[:, :w], e[:, :w], mask_sb[:, j, :w])
                    e_tiles.append(e)

                o_ps = acc_pool.tile([128, QR], F32, tag="outT")
                for kt in range(nkt):
                    nc.tensor.matmul(
                        o_ps[:],
                        v_nat[kt][:],
                        e_tiles[kt][:],
                        start=(kt == 0), stop=(kt == nkt - 1))
                l_ps = acc_pool.tile([1, QR], F32, tag="lacc")
                for kt in range(nkt):
                    nc.tensor.matmul(
                        l_ps[:],
                        ones_sb[:],
                        e_tiles[kt][:],
                        start=(kt == 0), stop=(kt == nkt - 1))

                ot = osmall_pool.tile([128, QR], F32, tag=f"outT{r}")
                nc.vector.tensor_copy(ot[:], o_ps[:])
                outT_sb.append(ot)
                ls = osmall_pool.tile([1, QR], F32, tag=f"l{r}")
                nc.scalar.copy(ls[:], l_ps[:])
                l_sb.append(ls)

            # ---- stage 3: finalize -----------------------------------------
            # lT: [1,512] rows -> [128, NQR*4] column layout via tiny matmuls
            lt_ps = ltps_pool.tile([128, 4 * NQR], F32, tag="ltps")
            for r in range(NQR):
                for u in range(4):
                    nc.tensor.matmul(
                        lt_ps[:, 4 * r + u:4 * r + u + 1],
                        l_sb[r][0:1, 128 * u:128 * (u + 1)],
                        ident_sb[0:1, 0:1],
                        start=True, stop=True)
            recip = osmall_pool.tile([128, 4 * NQR], F32, tag="recip")
            nc.vector.reciprocal(recip[:], lt_ps[:])

            for r in range(NQR):
                for u in range(4):
                    qt = 4 * r + u
                    ps = trps_pool.tile([128, 128], F32, tag="trps")
                    nc.tensor.transpose(
                        ps[:], outT_sb[r][:, 128 * u:128 * (u + 1)],
                        ident_sb[:])
                    of = ofin_pool.tile([128, 128], F32, tag="ofin")
                    nc.vector.tensor_scalar_mul(
                        of[:], ps[:], recip[:, qt:qt + 1])
                    nc.sync.dma_start(
                        out_d.ap()[128 * qt:128 * (qt + 1), :], of[:])

    nc.compile()
    return nc


_PROGRAM = None


def _get_program():
    global _PROGRAM
    if _PROGRAM is None:
        _PROGRAM = _build_program()
    return _PROGRAM


def _round_fp32r(a):
    b = np.ascontiguousarray(a, dtype=np.float32).view(np.uint32)
    lsb = (b >> 12) & 1
    r = (b + 0x7FF + lsb) & np.uint32(0xFFFFF000)
    return r.view(np.float32)


def _host_inputs(x, Wq, Wk, Wv):
    x = np.asarray(x, dtype=np.float32)
    Wq = np.ascontiguousarray(np.asarray(Wq, dtype=np.float32))
    Wk = np.ascontiguousarray(np.asarray(Wk, dtype=np.float32))
    Wv = np.ascontiguousarray(np.asarray(Wv, dtype=np.float32))

    # masks[j][pk, fq] = 1.0 iff allowed: fq >= 128*j + pk (within the
    # diagonal-straddling strip kt = 4r + j of q-range r)
    pk = np.arange(128)[:, None]
    fq = np.arange(QR)[None, :]
    masks = np.stack(
        [(fq >= 128 * j + pk).astype(np.float32) for j in range(4)])
    ident = np.eye(128, dtype=np.float32)
    ones = np.ones((128, 1), dtype=np.float32)

    in_maps = []
    for b in range(NCORES):
        in_maps.append({
            "xT": _round_fp32r(x[b].T),
            "Wq": _round_fp32r(Wq), "Wk": _round_fp32r(Wk), "Wv": _round_fp32r(Wv),
            "masks": masks, "ident": ident, "ones": ones,
        })
    return in_maps


def run(x, Wq, Wk, Wv, trace=False, **kwargs):
    nc = _get_program()
    in_maps = _host_inputs(x, Wq, Wk, Wv)
    res = run_bass_kernel_spmd(nc, in_maps, core_ids=list(range(NCORES)),
                               trace=trace, **kwargs)
    out = np.stack([res.results[b]["out"] for b in range(NCORES)], axis=0)
    return out.astype(np.float32), res


def kernel(x, Wq, Wk, Wv):
    out, _ = run(x, Wq, Wk, Wv)
    return out
